# revision 4
# baseline (speedup 1.0000x reference)
"""Trainium2 Bass kernel for nn_AdjacencyProcessing (8192x8192 adjacency
normalisation), distributed row-wise across 8 NeuronCores.

out[i,j] = s_i * A[i,j] + d_i * eye[i,j]
  rs_i = sum_j A[i,j]
  s_i  = 1 / (max(1, rs_i) * (rs_i + 1))
  d_i  = (1 + REG) / (rs_i + 1)

Each core processes a [1024, 8192] row shard: row sums and row scaling are
fully local. I/O is bf16 (well within the accuracy budget for uniform [0,1)
data) which halves HBM traffic; compute is fp32 internally. The tiny diagonal
correction d (1024 floats per core) is returned as a second output and
applied on the host.
"""
import numpy as np

N = 8192
NCORES = 8
ROWS = N // NCORES  # 1024 rows per core
P = 128             # SBUF partitions
NT = ROWS // P      # 8 tiles per core
REG = 0.001

_cached_nc = None


def _build():
    import concourse.bacc as bacc
    import concourse.mybir as mybir
    from concourse.tile import TileContext

    nc = bacc.Bacc("TRN2", target_bir_lowering=False, debug=False,
                   num_devices=NCORES)
    adj = nc.declare_dram_parameter("adjacency", [ROWS, N], mybir.dt.bfloat16,
                                    isOutput=False)
    out = nc.declare_dram_parameter("out", [ROWS, N], mybir.dt.bfloat16,
                                    isOutput=True)
    dvec = nc.declare_dram_parameter("dvec", [ROWS, 1], mybir.dt.float32,
                                     isOutput=True)
    with TileContext(nc) as tc:
        with tc.tile_pool(name="data", bufs=8) as pool, \
             tc.tile_pool(name="small", bufs=2 * NT) as spool:
            for i in range(NT):
                tile = pool.tile([P, N], mybir.dt.bfloat16)
                # loads on the SP HWDGE ring (nothing else on this engine)
                nc.sync.dma_start(out=tile[:], in_=adj[i * P:(i + 1) * P, :])
                # row sums on ACT: in-place identity copy with accumulate
                rs = spool.tile([P, 1], mybir.dt.float32, tag="rs")
                nc.scalar.activation(tile[:], tile[:],
                                     mybir.ActivationFunctionType.Copy,
                                     scale=1.0, accum_out=rs[:])
                m = spool.tile([P, 1], mybir.dt.float32, tag="m")
                nc.vector.tensor_scalar_max(m[:], rs[:], 1.0)
                denom = spool.tile([P, 1], mybir.dt.float32, tag="denom")
                nc.vector.tensor_scalar_add(denom[:], rs[:], 1.0)
                prod = spool.tile([P, 1], mybir.dt.float32, tag="prod")
                nc.vector.tensor_mul(prod[:], m[:], denom[:])
                s = spool.tile([P, 1], mybir.dt.float32, tag="s")
                nc.vector.reciprocal(s[:], prod[:])
                dn = spool.tile([P, 1], mybir.dt.float32, tag="dn")
                nc.vector.reciprocal(dn[:], denom[:])
                d = spool.tile([P, 1], mybir.dt.float32, tag="d")
                nc.vector.tensor_scalar_mul(d[:], dn[:], 1.0 + REG)
                # scale rows in place on DVE (bf16 tensor_scalar hits 4x mode)
                nc.vector.tensor_scalar_mul(tile[:], tile[:], s[:])
                # stores on the GPSIMD SWDGE ring: blocking waits are free there
                nc.gpsimd.dma_start(out=out[i * P:(i + 1) * P, :], in_=tile[:])
                nc.gpsimd.dma_start(out=dvec[i * P:(i + 1) * P, :], in_=d[:])
    nc.finalize()
    return nc


def run(adjacency: np.ndarray, trace: bool = False):
    """Run on 8 NeuronCores; returns (full_out, BassKernelResults)."""
    global _cached_nc
    import concourse.mybir as mybir
    from concourse.bass_utils import run_bass_kernel_spmd

    bf16 = mybir.dt.np(mybir.dt.bfloat16)
    adjacency = np.asarray(adjacency)
    assert adjacency.shape == (N, N)
    adj_bf16 = np.ascontiguousarray(adjacency.astype(bf16))
    if _cached_nc is None:
        _cached_nc = _build()
    in_maps = [{"adjacency": adj_bf16[c * ROWS:(c + 1) * ROWS]}
               for c in range(NCORES)]
    res = run_bass_kernel_spmd(_cached_nc, in_maps,
                               core_ids=list(range(NCORES)), trace=trace)
    full = np.empty((N, N), dtype=np.float32)
    dfull = np.empty(N, dtype=np.float32)
    for c in range(NCORES):
        full[c * ROWS:(c + 1) * ROWS] = res.results[c]["out"]
        dfull[c * ROWS:(c + 1) * ROWS] = res.results[c]["dvec"].reshape(-1)
    idx = np.arange(N)
    full[idx, idx] += dfull
    return full, res


def kernel(adjacency: np.ndarray) -> np.ndarray:
    out, _ = run(adjacency, trace=False)
    return out


# revision 6
# speedup vs baseline: 1.2342x; 1.2342x over previous
"""Trainium2 Bass kernel for nn_AdjacencyProcessing (8192x8192 adjacency
normalisation), distributed row-wise across 8 NeuronCores.

out[i,j] = s_i * A[i,j] + d_i * eye[i,j]
  rs_i = sum_j A[i,j]
  s_i  = 1 / (max(1, rs_i) * (rs_i + 1))
  d_i  = (1 + REG) / (rs_i + 1)

Each core processes a [1024, 8192] row shard: row sums and row scaling are
fully local. I/O is bf16 (well within the accuracy budget for uniform [0,1)
data) which halves HBM traffic; compute is fp32 internally. The tiny diagonal
correction d (1024 floats per core) is returned as a second output and
applied on the host.
"""
import numpy as np

N = 8192
NCORES = 8
ROWS = N // NCORES  # 1024 rows per core
P = 128             # SBUF partitions
NT = ROWS // P      # 8 tiles per core
REG = 0.001

_cached_nc = None


def _build():
    import concourse.bacc as bacc
    import concourse.mybir as mybir
    from concourse.tile import TileContext

    nc = bacc.Bacc("TRN2", target_bir_lowering=False, debug=False,
                   num_devices=NCORES)
    adj = nc.declare_dram_parameter("adjacency", [ROWS, N], mybir.dt.bfloat16,
                                    isOutput=False)
    out = nc.declare_dram_parameter("out", [ROWS, N], mybir.dt.bfloat16,
                                    isOutput=True)
    dvec = nc.declare_dram_parameter("dvec", [ROWS, 1], mybir.dt.float32,
                                     isOutput=True)
    with TileContext(nc) as tc:
        with tc.tile_pool(name="data", bufs=8) as pool, \
             tc.tile_pool(name="small", bufs=2 * NT) as spool:
            for i in range(NT):
                tile = pool.tile([P, N], mybir.dt.bfloat16)
                # loads on the SP HWDGE ring (nothing else on this engine)
                nc.sync.dma_start(out=tile[:], in_=adj[i * P:(i + 1) * P, :])
                # row sums on DVE
                rs = spool.tile([P, 1], mybir.dt.float32, tag="rs")
                nc.vector.reduce_sum(rs[:], tile[:], axis=mybir.AxisListType.X)
                m = spool.tile([P, 1], mybir.dt.float32, tag="m")
                nc.vector.tensor_scalar_max(m[:], rs[:], 1.0)
                denom = spool.tile([P, 1], mybir.dt.float32, tag="denom")
                nc.vector.tensor_scalar_add(denom[:], rs[:], 1.0)
                prod = spool.tile([P, 1], mybir.dt.float32, tag="prod")
                nc.vector.tensor_mul(prod[:], m[:], denom[:])
                s = spool.tile([P, 1], mybir.dt.float32, tag="s")
                nc.vector.reciprocal(s[:], prod[:])
                dn = spool.tile([P, 1], mybir.dt.float32, tag="dn")
                nc.vector.reciprocal(dn[:], denom[:])
                d = spool.tile([P, 1], mybir.dt.float32, tag="d")
                nc.vector.tensor_scalar_mul(d[:], dn[:], 1.0 + REG)
                # scale rows in place on DVE (bf16 tensor_scalar hits 4x mode)
                nc.vector.tensor_scalar_mul(tile[:], tile[:], s[:])
                # stores on the ACT HWDGE ring; ACT runs no compute, so its
                # sequencer blocking on ring-full costs nothing
                nc.scalar.dma_start(out=out[i * P:(i + 1) * P, :], in_=tile[:])
                nc.scalar.dma_start(out=dvec[i * P:(i + 1) * P, :], in_=d[:])
    nc.finalize()
    return nc


def run(adjacency: np.ndarray, trace: bool = False):
    """Run on 8 NeuronCores; returns (full_out, BassKernelResults)."""
    global _cached_nc
    import concourse.mybir as mybir
    from concourse.bass_utils import run_bass_kernel_spmd

    bf16 = mybir.dt.np(mybir.dt.bfloat16)
    adjacency = np.asarray(adjacency)
    assert adjacency.shape == (N, N)
    adj_bf16 = np.ascontiguousarray(adjacency.astype(bf16))
    if _cached_nc is None:
        _cached_nc = _build()
    in_maps = [{"adjacency": adj_bf16[c * ROWS:(c + 1) * ROWS]}
               for c in range(NCORES)]
    res = run_bass_kernel_spmd(_cached_nc, in_maps,
                               core_ids=list(range(NCORES)), trace=trace)
    full = np.empty((N, N), dtype=np.float32)
    dfull = np.empty(N, dtype=np.float32)
    for c in range(NCORES):
        full[c * ROWS:(c + 1) * ROWS] = res.results[c]["out"]
        dfull[c * ROWS:(c + 1) * ROWS] = res.results[c]["dvec"].reshape(-1)
    idx = np.arange(N)
    full[idx, idx] += dfull
    return full, res


def kernel(adjacency: np.ndarray) -> np.ndarray:
    out, _ = run(adjacency, trace=False)
    return out


# revision 7
# speedup vs baseline: 1.3167x; 1.0668x over previous
"""Trainium2 Bass kernel for nn_AdjacencyProcessing (8192x8192 adjacency
normalisation), distributed row-wise across 8 NeuronCores.

out[i,j] = s_i * A[i,j] + d_i * eye[i,j]
  rs_i = sum_j A[i,j]
  s_i  = 1 / (max(1, rs_i) * (rs_i + 1))
  d_i  = (1 + REG) / (rs_i + 1)

Each core processes a [1024, 8192] row shard: row sums and row scaling are
fully local. I/O is bf16 (well within the accuracy budget for uniform [0,1)
data) which halves HBM traffic; compute is fp32 internally. The tiny diagonal
correction d (1024 floats per core) is returned as a second output and
applied on the host.
"""
import numpy as np

N = 8192
NCORES = 8
ROWS = N // NCORES  # 1024 rows per core
P = 128             # SBUF partitions
NT = ROWS // P      # 8 tiles per core
REG = 0.001

_cached_nc = None


def _build():
    import concourse.bacc as bacc
    import concourse.mybir as mybir
    from concourse.tile import TileContext

    nc = bacc.Bacc("TRN2", target_bir_lowering=False, debug=False,
                   num_devices=NCORES)
    adj = nc.declare_dram_parameter("adjacency", [ROWS, N], mybir.dt.bfloat16,
                                    isOutput=False)
    out = nc.declare_dram_parameter("out", [ROWS, N], mybir.dt.bfloat16,
                                    isOutput=True)
    dvec = nc.declare_dram_parameter("dvec", [ROWS, 1], mybir.dt.float32,
                                     isOutput=True)
    with TileContext(nc) as tc:
        with tc.tile_pool(name="data", bufs=8) as pool, \
             tc.tile_pool(name="small", bufs=2 * NT) as spool:
            for i in range(NT):
                tile = pool.tile([P, N], mybir.dt.bfloat16)
                # loads on the SP HWDGE ring (nothing else on this engine)
                nc.sync.dma_start(out=tile[:], in_=adj[i * P:(i + 1) * P, :])
                # row sums alternate ACT (copy+accumulate) / DVE (reduce) so
                # neither engine falls behind the load stream
                rs = spool.tile([P, 1], mybir.dt.float32, tag="rs")
                if i % 2 == 0:
                    nc.scalar.activation(tile[:], tile[:],
                                         mybir.ActivationFunctionType.Copy,
                                         scale=1.0, accum_out=rs[:])
                else:
                    nc.vector.reduce_sum(rs[:], tile[:],
                                         axis=mybir.AxisListType.X)
                m = spool.tile([P, 1], mybir.dt.float32, tag="m")
                nc.vector.tensor_scalar_max(m[:], rs[:], 1.0)
                denom = spool.tile([P, 1], mybir.dt.float32, tag="denom")
                nc.vector.tensor_scalar_add(denom[:], rs[:], 1.0)
                prod = spool.tile([P, 1], mybir.dt.float32, tag="prod")
                nc.vector.tensor_mul(prod[:], m[:], denom[:])
                s = spool.tile([P, 1], mybir.dt.float32, tag="s")
                nc.vector.reciprocal(s[:], prod[:])
                dn = spool.tile([P, 1], mybir.dt.float32, tag="dn")
                nc.vector.reciprocal(dn[:], denom[:])
                d = spool.tile([P, 1], mybir.dt.float32, tag="d")
                nc.vector.tensor_scalar_mul(d[:], dn[:], 1.0 + REG)
                # scale rows in place on DVE (bf16 tensor_scalar hits 4x mode)
                nc.vector.tensor_scalar_mul(tile[:], tile[:], s[:])
                # stores on the ACT HWDGE ring; ACT runs no compute, so its
                # sequencer blocking on ring-full costs nothing
                nc.scalar.dma_start(out=out[i * P:(i + 1) * P, :], in_=tile[:])
                nc.scalar.dma_start(out=dvec[i * P:(i + 1) * P, :], in_=d[:])
    nc.finalize()
    return nc


def run(adjacency: np.ndarray, trace: bool = False):
    """Run on 8 NeuronCores; returns (full_out, BassKernelResults)."""
    global _cached_nc
    import concourse.mybir as mybir
    from concourse.bass_utils import run_bass_kernel_spmd

    bf16 = mybir.dt.np(mybir.dt.bfloat16)
    adjacency = np.asarray(adjacency)
    assert adjacency.shape == (N, N)
    adj_bf16 = np.ascontiguousarray(adjacency.astype(bf16))
    if _cached_nc is None:
        _cached_nc = _build()
    in_maps = [{"adjacency": adj_bf16[c * ROWS:(c + 1) * ROWS]}
               for c in range(NCORES)]
    res = run_bass_kernel_spmd(_cached_nc, in_maps,
                               core_ids=list(range(NCORES)), trace=trace)
    full = np.empty((N, N), dtype=np.float32)
    dfull = np.empty(N, dtype=np.float32)
    for c in range(NCORES):
        full[c * ROWS:(c + 1) * ROWS] = res.results[c]["out"]
        dfull[c * ROWS:(c + 1) * ROWS] = res.results[c]["dvec"].reshape(-1)
    idx = np.arange(N)
    full[idx, idx] += dfull
    return full, res


def kernel(adjacency: np.ndarray) -> np.ndarray:
    out, _ = run(adjacency, trace=False)
    return out


# revision 10
# speedup vs baseline: 1.3798x; 1.0479x over previous
"""Trainium2 Bass kernel for nn_AdjacencyProcessing (8192x8192 adjacency
normalisation), distributed row-wise across 8 NeuronCores.

out[i,j] = s_i * A[i,j] + d_i * eye[i,j]
  rs_i = sum_j A[i,j]
  s_i  = 1 / (max(1, rs_i) * (rs_i + 1))
  d_i  = (1 + REG) / (rs_i + 1)

Each core processes a [1024, 8192] row shard: row sums and row scaling are
fully local. I/O is bf16 (well within the accuracy budget for uniform [0,1)
data) which halves HBM traffic; compute is fp32 internally. The tiny diagonal
correction d (1024 floats per core) is returned as a second output and
applied on the host.
"""
import numpy as np

N = 8192
NCORES = 8
ROWS = N // NCORES  # 1024 rows per core
P = 128             # SBUF partitions
NT = ROWS // P      # 8 tiles per core
REG = 0.001

_cached_nc = None


def _build():
    import concourse.bacc as bacc
    import concourse.mybir as mybir
    from concourse.tile import TileContext

    nc = bacc.Bacc("TRN2", target_bir_lowering=False, debug=False,
                   num_devices=NCORES)
    adj = nc.declare_dram_parameter("adjacency", [ROWS, N], mybir.dt.bfloat16,
                                    isOutput=False)
    out = nc.declare_dram_parameter("out", [ROWS, N], mybir.dt.bfloat16,
                                    isOutput=True)
    dvec = nc.declare_dram_parameter("dvec", [ROWS, 1], mybir.dt.float32,
                                     isOutput=True)
    with TileContext(nc) as tc:
        with tc.tile_pool(name="data", bufs=NT) as pool, \
             tc.tile_pool(name="small", bufs=2 * NT) as spool:
            tiles, ds = [], []
            # Phase 1: prefetch every tile on the SP HWDGE ring. With one
            # buffer per tile, no load ever waits, and the later stores queue
            # strictly behind the loads in the same FIFO.
            for i in range(NT):
                tile = pool.tile([P, N], mybir.dt.bfloat16)
                nc.sync.dma_start(out=tile[:], in_=adj[i * P:(i + 1) * P, :])
                tiles.append(tile)
            # Phase 2: per-tile compute. Row sums alternate ACT
            # (copy+accumulate) / DVE (tensor_reduce) so reduce throughput
            # matches load arrival; the small chain and the bf16 4x-mode
            # scale run on DVE.
            for i in range(NT):
                tile = tiles[i]
                rs = spool.tile([P, 1], mybir.dt.float32, tag="rs")
                if i % 2 == 0:
                    nc.scalar.activation(tile[:], tile[:],
                                         mybir.ActivationFunctionType.Copy,
                                         scale=1.0, accum_out=rs[:])
                else:
                    nc.vector.reduce_sum(rs[:], tile[:],
                                         axis=mybir.AxisListType.X)
                m = spool.tile([P, 1], mybir.dt.float32, tag="m")
                nc.vector.tensor_scalar_max(m[:], rs[:], 1.0)
                denom = spool.tile([P, 1], mybir.dt.float32, tag="denom")
                nc.vector.tensor_scalar_add(denom[:], rs[:], 1.0)
                prod = spool.tile([P, 1], mybir.dt.float32, tag="prod")
                nc.vector.tensor_mul(prod[:], m[:], denom[:])
                s = spool.tile([P, 1], mybir.dt.float32, tag="s")
                nc.vector.reciprocal(s[:], prod[:])
                dn = spool.tile([P, 1], mybir.dt.float32, tag="dn")
                nc.vector.reciprocal(dn[:], denom[:])
                d = spool.tile([P, 1], mybir.dt.float32, tag="d")
                nc.vector.tensor_scalar_mul(d[:], dn[:], 1.0 + REG)
                ds.append(d)
                # scale rows in place on DVE (bf16 tensor_scalar hits 4x mode)
                nc.vector.tensor_scalar_mul(tile[:], tile[:], s[:])
            # Phase 3: stores, also on the SP ring — FIFO-ordered behind all
            # loads; by the time the ring reaches store i, scale i is done.
            for i in range(NT):
                nc.sync.dma_start(out=out[i * P:(i + 1) * P, :],
                                  in_=tiles[i][:])
                nc.sync.dma_start(out=dvec[i * P:(i + 1) * P, :], in_=ds[i][:])
    nc.finalize()
    return nc


def run(adjacency: np.ndarray, trace: bool = False):
    """Run on 8 NeuronCores; returns (full_out, BassKernelResults)."""
    global _cached_nc
    import concourse.mybir as mybir
    from concourse.bass_utils import run_bass_kernel_spmd

    bf16 = mybir.dt.np(mybir.dt.bfloat16)
    adjacency = np.asarray(adjacency)
    assert adjacency.shape == (N, N)
    adj_bf16 = np.ascontiguousarray(adjacency.astype(bf16))
    if _cached_nc is None:
        _cached_nc = _build()
    in_maps = [{"adjacency": adj_bf16[c * ROWS:(c + 1) * ROWS]}
               for c in range(NCORES)]
    res = run_bass_kernel_spmd(_cached_nc, in_maps,
                               core_ids=list(range(NCORES)), trace=trace)
    full = np.empty((N, N), dtype=np.float32)
    dfull = np.empty(N, dtype=np.float32)
    for c in range(NCORES):
        full[c * ROWS:(c + 1) * ROWS] = res.results[c]["out"]
        dfull[c * ROWS:(c + 1) * ROWS] = res.results[c]["dvec"].reshape(-1)
    idx = np.arange(N)
    full[idx, idx] += dfull
    return full, res


def kernel(adjacency: np.ndarray) -> np.ndarray:
    out, _ = run(adjacency, trace=False)
    return out


# revision 14
# speedup vs baseline: 1.6825x; 1.2194x over previous
"""Trainium2 Bass kernel for nn_AdjacencyProcessing (8192x8192 adjacency
normalisation), distributed row-wise across 8 NeuronCores.

out[i,j] = s_i * A[i,j] + d_i * eye[i,j]
  rs_i = sum_j A[i,j]
  s_i  = 1 / (max(1, rs_i) * (rs_i + 1))
  d_i  = (1 + REG) / (rs_i + 1)

Each core processes a [1024, 8192] row shard: row sums and row scaling are
fully local. I/O is bf16 (well within the accuracy budget for uniform [0,1)
data) which halves HBM traffic; compute is fp32 internally. The tiny diagonal
correction d (1024 floats per core) is returned as a second output and
applied on the host.
"""
import numpy as np

N = 8192
NCORES = 8
ROWS = N // NCORES  # 1024 rows per core
P = 128             # SBUF partitions
NT = ROWS // P      # 8 tiles per core
REG = 0.001

_cached_nc = None


def _build():
    import concourse.bacc as bacc
    import concourse.mybir as mybir
    from concourse.tile import TileContext

    nc = bacc.Bacc("TRN2", target_bir_lowering=False, debug=False,
                   num_devices=NCORES)
    adj = nc.declare_dram_parameter("adjacency", [ROWS, N], mybir.dt.bfloat16,
                                    isOutput=False)
    out = nc.declare_dram_parameter("out", [ROWS, N], mybir.dt.bfloat16,
                                    isOutput=True)
    # d values laid out [partition, tile]: global row = t*128 + p (host remaps)
    dvec = nc.declare_dram_parameter("dvec", [P, NT], mybir.dt.float32,
                                     isOutput=True)
    with TileContext(nc) as tc:
        with tc.tile_pool(name="data", bufs=NT) as pool, \
             tc.tile_pool(name="small", bufs=2 * NT) as spool, \
             tc.tile_pool(name="dpool", bufs=1) as dpool:
            tiles = []
            dtile = dpool.tile([P, NT], mybir.dt.float32)
            # Phase 1: prefetch every tile on the SP HWDGE ring. With one
            # buffer per tile, no load ever waits, and the later stores queue
            # strictly behind the loads in the same FIFO.
            for i in range(NT):
                tile = pool.tile([P, N], mybir.dt.bfloat16)
                nc.sync.dma_start(out=tile[:], in_=adj[i * P:(i + 1) * P, :])
                tiles.append(tile)
            # Phase 2: per-tile compute. Row sums alternate ACT
            # (copy+accumulate) / DVE (tensor_reduce) so reduce throughput
            # matches load arrival; the small chain and the bf16 4x-mode
            # scale run on DVE.
            for i in range(NT):
                tile = tiles[i]
                rs = spool.tile([P, 1], mybir.dt.float32, tag="rs")
                if i % 2 == 0:
                    nc.scalar.activation(tile[:], tile[:],
                                         mybir.ActivationFunctionType.Copy,
                                         scale=1.0, accum_out=rs[:])
                else:
                    nc.vector.reduce_sum(rs[:], tile[:],
                                         axis=mybir.AxisListType.X)
                m = spool.tile([P, 1], mybir.dt.float32, tag="m")
                nc.vector.tensor_scalar_max(m[:], rs[:], 1.0)
                denom = spool.tile([P, 1], mybir.dt.float32, tag="denom")
                nc.vector.tensor_scalar_add(denom[:], rs[:], 1.0)
                prod = spool.tile([P, 1], mybir.dt.float32, tag="prod")
                nc.vector.tensor_mul(prod[:], m[:], denom[:])
                s = spool.tile([P, 1], mybir.dt.float32, tag="s")
                nc.vector.reciprocal(s[:], prod[:])
                dn = spool.tile([P, 1], mybir.dt.float32, tag="dn")
                nc.vector.reciprocal(dn[:], denom[:])
                nc.vector.tensor_scalar_mul(dtile[:, i:i + 1], dn[:],
                                            1.0 + REG)
                # scale rows in place on DVE (bf16 tensor_scalar hits 4x mode)
                nc.vector.tensor_scalar_mul(tile[:], tile[:], s[:])
            # Phase 3: stores, also on the SP ring — FIFO-ordered behind all
            # loads; by the time the ring reaches store i, scale i is done.
            for i in range(NT):
                nc.sync.dma_start(out=out[i * P:(i + 1) * P, :],
                                  in_=tiles[i][:])
            # single tiny d store on the otherwise-idle ACT ring
            nc.scalar.dma_start(out=dvec[:, :], in_=dtile[:])
    nc.finalize()
    return nc


def run(adjacency: np.ndarray, trace: bool = False):
    """Run on 8 NeuronCores; returns (full_out, BassKernelResults)."""
    global _cached_nc
    import concourse.mybir as mybir
    from concourse.bass_utils import run_bass_kernel_spmd

    bf16 = mybir.dt.np(mybir.dt.bfloat16)
    adjacency = np.asarray(adjacency)
    assert adjacency.shape == (N, N)
    adj_bf16 = np.ascontiguousarray(adjacency.astype(bf16))
    if _cached_nc is None:
        _cached_nc = _build()
    in_maps = [{"adjacency": adj_bf16[c * ROWS:(c + 1) * ROWS]}
               for c in range(NCORES)]
    res = run_bass_kernel_spmd(_cached_nc, in_maps,
                               core_ids=list(range(NCORES)), trace=trace)
    full = np.empty((N, N), dtype=np.float32)
    dfull = np.empty(N, dtype=np.float32)
    for c in range(NCORES):
        full[c * ROWS:(c + 1) * ROWS] = res.results[c]["out"]
        # dvec comes back [P, NT]: global row within shard = t*P + p
        dfull[c * ROWS:(c + 1) * ROWS] = res.results[c]["dvec"].T.reshape(-1)
    idx = np.arange(N)
    full[idx, idx] += dfull
    return full, res


def kernel(adjacency: np.ndarray) -> np.ndarray:
    out, _ = run(adjacency, trace=False)
    return out


# revision 15
# speedup vs baseline: 1.8780x; 1.1161x over previous
"""Trainium2 Bass kernel for nn_AdjacencyProcessing (8192x8192 adjacency
normalisation), distributed row-wise across 8 NeuronCores.

out[i,j] = s_i * A[i,j] + d_i * eye[i,j]
  rs_i = sum_j A[i,j]
  s_i  = 1 / (max(1, rs_i) * (rs_i + 1))
  d_i  = (1 + REG) / (rs_i + 1)

Each core processes a [1024, 8192] row shard: row sums and row scaling are
fully local; the eye addition targets the local diagonal block, whose column
offset comes from partition_id() at runtime (SPMD-uniform program). I/O is
bf16 (well within the accuracy budget for uniform [0,1) data) which halves
HBM traffic; compute is fp32 internally.
"""
import numpy as np

N = 8192
NCORES = 8
ROWS = N // NCORES  # 1024 rows per core
P = 128             # SBUF partitions
NT = ROWS // P      # 8 tiles per core
REG = 0.001

_cached_nc = None


def _build():
    import concourse.bass as bass
    import concourse.bacc as bacc
    import concourse.mybir as mybir
    from concourse.tile import TileContext

    nc = bacc.Bacc("TRN2", target_bir_lowering=False, debug=False,
                   num_devices=NCORES)
    adj = nc.declare_dram_parameter("adjacency", [ROWS, N], mybir.dt.bfloat16,
                                    isOutput=False)
    eye = nc.declare_dram_parameter("eye", [P, P], mybir.dt.bfloat16,
                                    isOutput=False)
    out = nc.declare_dram_parameter("out", [ROWS, N], mybir.dt.bfloat16,
                                    isOutput=True)
    with TileContext(nc) as tc:
        with tc.tile_pool(name="data", bufs=NT) as pool, \
             tc.tile_pool(name="small", bufs=2 * NT) as spool, \
             tc.tile_pool(name="eyep", bufs=1) as eyep:
            eyet = eyep.tile([P, P], mybir.dt.bfloat16)
            nc.sync.dma_start(out=eyet[:], in_=eye[:, :])
            tiles = []
            # Phase 1: prefetch every tile on the SP HWDGE ring. With one
            # buffer per tile, no load ever waits, and the later stores queue
            # strictly behind the loads in the same FIFO.
            for i in range(NT):
                tile = pool.tile([P, N], mybir.dt.bfloat16)
                nc.sync.dma_start(out=tile[:], in_=adj[i * P:(i + 1) * P, :])
                tiles.append(tile)
            # Phase 2: per-tile compute. Row sums alternate ACT
            # (copy+accumulate) / DVE (tensor_reduce) so reduce throughput
            # matches load arrival; the small chain and the bf16 4x-mode
            # scale run on DVE.
            pid = nc.vector.partition_id()
            for i in range(NT):
                tile = tiles[i]
                rs = spool.tile([P, 1], mybir.dt.float32, tag="rs")
                if i % 2 == 0:
                    nc.scalar.activation(tile[:], tile[:],
                                         mybir.ActivationFunctionType.Copy,
                                         scale=1.0, accum_out=rs[:])
                else:
                    nc.vector.reduce_sum(rs[:], tile[:],
                                         axis=mybir.AxisListType.X)
                m = spool.tile([P, 1], mybir.dt.float32, tag="m")
                nc.vector.tensor_scalar_max(m[:], rs[:], 1.0)
                denom = spool.tile([P, 1], mybir.dt.float32, tag="denom")
                nc.vector.tensor_scalar_add(denom[:], rs[:], 1.0)
                prod = spool.tile([P, 1], mybir.dt.float32, tag="prod")
                nc.vector.tensor_mul(prod[:], m[:], denom[:])
                s = spool.tile([P, 1], mybir.dt.float32, tag="s")
                nc.vector.reciprocal(s[:], prod[:])
                dn = spool.tile([P, 1], mybir.dt.float32, tag="dn")
                nc.vector.reciprocal(dn[:], denom[:])
                d = spool.tile([P, 1], mybir.dt.float32, tag="d")
                nc.vector.tensor_scalar_mul(d[:], dn[:], 1.0 + REG)
                # scale rows in place on DVE (bf16 tensor_scalar hits 4x mode)
                nc.vector.tensor_scalar_mul(tile[:], tile[:], s[:])
                # diagonal: add d*eye into the local diagonal block, at the
                # runtime column offset (pid*NT + i) * P
                eyed = spool.tile([P, P], mybir.dt.bfloat16, tag="eyed")
                nc.vector.tensor_scalar_mul(eyed[:], eyet[:], d[:])
                dyn = bass.ts(pid * NT + i, P)
                nc.vector.tensor_add(tile[:, dyn], tile[:, dyn], eyed[:])
            # Phase 3: stores, also on the SP ring — FIFO-ordered behind all
            # loads; by the time the ring reaches store i, its data is ready.
            for i in range(NT):
                nc.sync.dma_start(out=out[i * P:(i + 1) * P, :],
                                  in_=tiles[i][:])
    nc.finalize()
    return nc


def run(adjacency: np.ndarray, trace: bool = False):
    """Run on 8 NeuronCores; returns (full_out, BassKernelResults)."""
    global _cached_nc
    import concourse.mybir as mybir
    from concourse.bass_utils import run_bass_kernel_spmd

    bf16 = mybir.dt.np(mybir.dt.bfloat16)
    adjacency = np.asarray(adjacency)
    assert adjacency.shape == (N, N)
    adj_bf16 = np.ascontiguousarray(adjacency.astype(bf16))
    eye = np.eye(P, dtype=bf16)
    if _cached_nc is None:
        _cached_nc = _build()
    in_maps = [{"adjacency": adj_bf16[c * ROWS:(c + 1) * ROWS], "eye": eye}
               for c in range(NCORES)]
    res = run_bass_kernel_spmd(_cached_nc, in_maps,
                               core_ids=list(range(NCORES)), trace=trace)
    full = np.empty((N, N), dtype=np.float32)
    for c in range(NCORES):
        full[c * ROWS:(c + 1) * ROWS] = res.results[c]["out"]
    return full, res


def kernel(adjacency: np.ndarray) -> np.ndarray:
    out, _ = run(adjacency, trace=False)
    return out
